# revision 1
# baseline (speedup 1.0000x reference)
"""Trainium2 Bass kernel for ConditionalLinearAttention.

Math (per batch element b, shapes hardcoded):
  xf  = x[b].reshape(256, 4096)
  cf  = cond_emb[b].reshape(512, 128)
  kv  = Wcond @ cf                      # (1024, 128)
  k   = softmax(kv[:512], per-row over the 128 cond positions)
  v   = kv[512:]
  ctx[h] = k_h @ v_h.T                  # (64, 64) per head h
  out = Wout @ apply(ctx) @ Wq @ xf + b_out

ctx is tiny and per-batch, so the whole attention folds into one per-batch
matrix W_comb = Wout @ ctxE @ Wq (256x256); the spatial dimension then sees
ONE (256x256)@(256x4096) GEMM. Sharding: data-parallel over batch, one
batch element per core.

v2 design notes (all driven by perfetto traces of the v1 kernel):
  * everything bf16 on the wire (5.9 MB/core vs 11.8 fp32); DMA engines
    move bytes at ~25.6 GB/s/queue x 16, so bytes are the binding resource.
  * DMA trigger instructions (DIRECT2D) cost ~0.4-0.7us each on the
    issuing sequencer and descriptors fan out across the 16 engines, so
    all phase-1 operands are HOST-PACKED into per-partition-contiguous
    tensors: one trigger = 128 descriptors of 3-8KB.
  * x and out live in parity-interleaved layouts (channel c = 2p+ck on
    partition p) so each DMA line is a 4KB/2KB contiguous block. The
    interleave is produced by host-side column permutations of Wq/Wout,
    which commute through the on-chip algebra for free.
  * softmax denominator is fused into the context matmul: ctx_and_Z =
    expkT_i^T @ [vT_i | ones] (N=129), killing 4 tiny matmuls + handoffs.
  * phase-1 chain is pipelined per head-pair across vector+scalar, and the
    1/Z scaling rides the diagonal-block extraction.
  * junk matmuls at the head keep the PE busy so HAM unthrottles the
    clock (1.2 -> 2.4 GHz) before the real dependent chain runs.
"""

import os

import numpy as np

B = 8
C = 256
N_SPATIAL = 4096  # 64*64
P = 128
N_CORES = 8

WARM = int(os.environ.get("KERNEL_WARM", "8"))  # PE warmup matmuls at head
WARM_MID = int(os.environ.get("KERNEL_WARM_MID", "0"))  # mid-chain keepalive

_CACHE = {}
LAST_RESULTS = None  # BassKernelResults of the most recent run (for test.py)


def _build_nc():
    import concourse.bacc as bacc
    import concourse.mybir as mybir
    import concourse.tile as tile

    fp32 = mybir.dt.float32
    bf16 = mybir.dt.bfloat16
    AF = mybir.ActivationFunctionType

    nc = bacc.Bacc("TRN2", target_bir_lowering=False, debug=False,
                   num_devices=N_CORES)

    # pack1: cf (4x128 cols) + wck (4x512)                   -> [128, 2560]
    # pack2: wcv (4x512)                                      -> [128, 2048]
    # pack3: wq_perm (4x256) + woT_perm (4x256)               -> [128, 2048]
    # xp:     x interleaved c=2p+ck, 4 col-chunks of 1024     -> [128,4,2,1024]
    # bias2:  b_out[2p+mo]                                    -> [128, 2] f32
    # outp:   out rows 2p+mo, 8 col-chunks of 512             -> [128,8,2,512]
    p1_t = nc.dram_tensor("p1", [P, 2560], bf16, kind="ExternalInput").ap()
    p2_t = nc.dram_tensor("p2", [P, 2048], bf16, kind="ExternalInput").ap()
    p3_t = nc.dram_tensor("p3", [P, 2048], bf16, kind="ExternalInput").ap()
    xp_t = nc.dram_tensor("xp", [P, 4, 2, 1024], bf16, kind="ExternalInput").ap()
    bias_t = nc.dram_tensor("bias2", [P, 2], fp32, kind="ExternalInput").ap()
    out_t = nc.dram_tensor("out", [P, 8, 2, 512], bf16, kind="ExternalOutput").ap()

    with tile.TileContext(nc) as tc:
        with (
            tc.tile_pool(name="main", bufs=1) as mainp,
            tc.tile_pool(name="work", bufs=2) as workp,
            tc.tile_pool(name="outp", bufs=6) as outp,
            tc.tile_pool(name="ps", bufs=2, space="PSUM") as psp,
            tc.tile_pool(name="psA", bufs=3, space="PSUM") as psA,
            tc.tile_pool(name="psO", bufs=3, space="PSUM") as psO,
        ):
            # --- junk-matmul operand, first so the PE can warm immediately
            wl = mainp.tile([P, 512], bf16)
            nc.gpsimd.memset(wl, 0.0)

            # --- input DMA triggers, critical-path order, all on sync HWDGE
            p1_sb = mainp.tile([P, 2560], bf16)
            nc.sync.dma_start(p1_sb, p1_t)
            p2_sb = mainp.tile([P, 2048], bf16)
            nc.sync.dma_start(p2_sb, p2_t)
            p3_sb = mainp.tile([P, 2048], bf16)
            nc.sync.dma_start(p3_sb, p3_t)
            x_sb = []
            for cc in range(4):
                t = mainp.tile([P, 2, 1024], bf16, tag=f"x{cc}")
                nc.sync.dma_start(t, xp_t[:, cc, :, :])
                x_sb.append(t)
            # small stuff off the sync ring
            bias_sb = mainp.tile([P, 2], fp32)
            nc.gpsimd.dma_start(bias_sb, bias_t)

            # persistent SBUF tiles + zero-fills (off critical path, gpsimd)
            vTo = mainp.tile([P, 4, 129], bf16)
            nc.gpsimd.memset(vTo[:, :, 128:129], 1.0)
            ctx_bd = mainp.tile([P, 4, 128], bf16)
            nc.gpsimd.memset(ctx_bd, 0.0)

            def keep_warm(n):
                for _ in range(n):
                    pj = psO.tile([P, 512], fp32, tag="O")
                    nc.tensor.matmul(pj, wl[:, 0:128], wl, start=True, stop=True)

            keep_warm(WARM)

            # --- phase 1: per-batch W_comb (256x256) ---
            # kvT k-half (cond position m on partitions)
            pkv = psp.tile([P, 512], fp32, tag="p1")
            for j in range(4):
                nc.tensor.matmul(pkv, p1_sb[:, 128 * j:128 * (j + 1)],
                                 p1_sb[:, 512 + 512 * j:512 + 512 * (j + 1)],
                                 start=(j == 0), stop=(j == 3))
            expkT = mainp.tile([P, 512], bf16)
            nc.scalar.activation(out=expkT, in_=pkv, func=AF.Exp)

            # kvT v-half
            pvv = psp.tile([P, 4, 128], fp32, tag="p1")
            for j in range(4):
                nc.tensor.matmul(pvv, p1_sb[:, 128 * j:128 * (j + 1)],
                                 p2_sb[:, 512 * j:512 * (j + 1)],
                                 start=(j == 0), stop=(j == 3))
            nc.vector.tensor_copy(out=vTo[:, 0:2, 0:128], in_=pvv[:, 0:2, :])
            nc.scalar.activation(out=vTo[:, 2:4, 0:128], in_=pvv[:, 2:4, :],
                                 func=AF.Identity)
            keep_warm(WARM_MID)

            # fused context + softmax denominator per head pair i:
            #   pc_i[:, 0:128] = expkT_i^T @ vT_i ; pc_i[:, 128] = Z
            # then extract diagonal 64x64 blocks scaled by 1/Z (low half on
            # vector, high half on scalar), pipelined with the A matmuls.
            pcs = []
            for i in range(4):
                pc = psA.tile([P, 129], fp32, tag="pA")
                nc.tensor.matmul(pc, expkT[:, 128 * i:128 * (i + 1)],
                                 vTo[:, i, :], start=True, stop=True)
                pcs.append(pc)
            A_sb = mainp.tile([P, 4, 256], bf16)
            for i in range(4):
                rc = workp.tile([P, 1], fp32, tag=f"r{i}")
                nc.vector.reciprocal(rc, pcs[i][:, 128:129])
                nc.vector.tensor_scalar_mul(ctx_bd[0:64, i, 0:64],
                                            pcs[i][0:64, 0:64], rc[0:64])
                nc.scalar.activation(out=ctx_bd[64:128, i, 64:128],
                                     in_=pcs[i][64:128, 64:128],
                                     func=AF.Identity, scale=rc[64:128])
                pa = psA.tile([P, 256], fp32, tag="pA")
                nc.tensor.matmul(pa, ctx_bd[:, i, :],
                                 p3_sb[:, 256 * i:256 * (i + 1)],
                                 start=True, stop=True)
                if i % 2 == 0:
                    nc.vector.tensor_copy(out=A_sb[:, i, :], in_=pa)
                else:
                    nc.scalar.activation(out=A_sb[:, i, :], in_=pa,
                                         func=AF.Identity)

            # W_combT[c, o'] = sum_he A[he, c] * WoutT_perm[he, o']
            wo_off = 1024
            wc_sb = mainp.tile([P, 2, 256], bf16)
            for ck in range(2):
                pw = psA.tile([P, 256], fp32, tag="pA")
                for kk in range(4):
                    nc.tensor.matmul(pw, A_sb[:, kk, 128 * ck:128 * (ck + 1)],
                                     p3_sb[:, wo_off + 256 * kk:wo_off + 256 * (kk + 1)],
                                     start=(kk == 0), stop=(kk == 3))
                if ck == 0:
                    nc.vector.tensor_copy(out=wc_sb[:, ck, :], in_=pw)
                else:
                    nc.scalar.activation(out=wc_sb[:, ck, :], in_=pw,
                                         func=AF.Identity)

            # --- phase 2: OUT = W_comb @ xf + bias, streamed over x chunks.
            # The last spatial tile is split in half so the final
            # compute->store->drain tail is shorter.
            tiles = [(nt, 0, 512) for nt in range(7)]
            tiles += [(7, 0, 256), (7, 256, 256)]
            for nt, c0, cw in tiles:
                cc, sub = nt // 2, nt % 2
                ot = outp.tile([P, 2, 512], bf16, tag="osb")
                for mo in range(2):
                    po = psO.tile([P, 512], fp32, tag="O")
                    for ck in range(2):
                        nc.tensor.matmul(
                            po[:, 0:cw], wc_sb[:, ck, 128 * mo:128 * (mo + 1)],
                            x_sb[cc][:, ck, 512 * sub + c0:512 * sub + c0 + cw],
                            start=(ck == 0), stop=(ck == 1))
                    if mo == 0:
                        nc.scalar.activation(out=ot[:, mo, 0:cw], in_=po[:, 0:cw],
                                             func=AF.Identity,
                                             bias=bias_sb[:, 0:1], scale=1.0)
                    else:
                        nc.vector.tensor_scalar_add(out=ot[:, mo, 0:cw],
                                                    in0=po[:, 0:cw],
                                                    scalar1=bias_sb[:, 1:2])
                nc.sync.dma_start(out_t[:, nt, :, c0:c0 + cw], ot[:, :, 0:cw])

    nc.compile()
    return nc


def kernel(x, cond_emb, Wq, Wcond, Wout, b_out):
    import ml_dtypes
    from concourse.bass_utils import run_bass_kernel_spmd

    global LAST_RESULTS

    if "nc" not in _CACHE:
        _CACHE["nc"] = _build_nc()
    nc = _CACHE["nc"]

    bf = ml_dtypes.bfloat16

    # cf chunks: cf[j*128+p, m] -> [p, j*128+m]
    cf = np.asarray(cond_emb, np.float32).reshape(B, 4, P, P)
    cf_p = np.transpose(cf, (0, 2, 1, 3)).reshape(B, P, 512)
    # wcondT chunks: wct[j*128+p, o] -> [p, j, o]
    wct = np.ascontiguousarray(np.asarray(Wcond, np.float32).T).reshape(4, P, 1024)
    wck = np.transpose(wct[:, :, 0:512], (1, 0, 2)).reshape(P, 2048)
    wcv = np.transpose(wct[:, :, 512:1024], (1, 0, 2)).reshape(P, 2048)
    # Wq with columns parity-permuted (c = 2j+ck -> block ck, col j)
    wq_perm = np.asarray(Wq, np.float32).reshape(512, P, 2)
    wq_perm = np.transpose(wq_perm, (0, 2, 1)).reshape(512, 256)
    wq_p = np.transpose(wq_perm.reshape(4, P, 256), (1, 0, 2)).reshape(P, 1024)
    # WoutT with columns parity-permuted (o = 2i+mo -> block mo, col i)
    woT = np.ascontiguousarray(np.asarray(Wout, np.float32).T)
    woT_perm = np.transpose(woT.reshape(512, P, 2), (0, 2, 1)).reshape(512, 256)
    wo_p = np.transpose(woT_perm.reshape(4, P, 256), (1, 0, 2)).reshape(P, 1024)

    p1 = np.empty((B, P, 2560), bf)
    p1[:, :, 0:512] = cf_p.astype(bf)
    p1[:, :, 512:2560] = wck[None].astype(bf)
    p2 = np.broadcast_to(wcv.astype(bf), (B, P, 2048))
    p3 = np.empty((P, 2048), bf)
    p3[:, 0:1024] = wq_p.astype(bf)
    p3[:, 1024:2048] = wo_p.astype(bf)
    p3 = np.broadcast_to(p3, (B, P, 2048))
    # x interleaved: xp[p, cc, ck, u] = x[b, 2p+ck, cc*1024+u]
    xr = np.asarray(x, np.float32).reshape(B, P, 2, 4, 1024)
    xp = np.transpose(xr, (0, 1, 3, 2, 4)).astype(bf)
    bias2 = np.ascontiguousarray(
        np.asarray(b_out, np.float32).reshape(P, 2))

    in_maps = [
        {
            "p1": np.ascontiguousarray(p1[b]),
            "p2": np.ascontiguousarray(p2[b]),
            "p3": np.ascontiguousarray(p3[b]),
            "xp": np.ascontiguousarray(xp[b]),
            "bias2": bias2,
        }
        for b in range(B)
    ]

    trace = bool(int(os.environ.get("KERNEL_TRACE", "0")))
    res = run_bass_kernel_spmd(nc, in_maps, core_ids=list(range(N_CORES)),
                               trace=trace)
    LAST_RESULTS = res
    # out_p[p, nt, mo, u] = out[b, 2p+mo, nt*512+u]
    outs = []
    for b in range(B):
        arr = np.asarray(res.results[b]["out"]).astype(np.float32)
        outs.append(np.transpose(arr, (0, 2, 1, 3)).reshape(C, N_SPATIAL))
    return np.stack(outs).reshape(B, C, 64, 64)


if __name__ == "__main__":
    xs = np.random.RandomState(0)
    ins = {
        "x": xs.randn(8, 256, 64, 64).astype(np.float32),
        "cond_emb": xs.randn(8, 512, 1, 128).astype(np.float32),
        "Wq": (xs.randn(512, 256) * 0.05).astype(np.float32),
        "Wcond": (xs.randn(1024, 512) * 0.05).astype(np.float32),
        "Wout": (xs.randn(256, 512) * 0.05).astype(np.float32),
        "b_out": np.zeros(256, np.float32),
    }
    o = kernel(**ins)
    print("ran, shape", o.shape)



# revision 2
# speedup vs baseline: 1.0303x; 1.0303x over previous
"""Trainium2 Bass kernel for ConditionalLinearAttention (v3).

Math (per batch element b, shapes hardcoded):
  xf  = x[b].reshape(256, 4096)
  cf  = cond_emb[b].reshape(512, 128)
  kv  = Wcond @ cf                      # (1024, 128)
  k   = softmax(kv[:512], per-row over the 128 cond positions)
  v   = kv[512:]
  ctx[h] = k_h @ v_h.T                  # (64, 64) per head h
  out = Wout @ apply(ctx) @ Wq @ xf + b_out

ctx is tiny and per-batch, so the whole attention folds into one per-batch
matrix W_comb = Wout @ ctxE @ Wq (256x256); the spatial dimension then sees
ONE (256x256)@(256x4096) GEMM. Sharding: data-parallel over batch, one
batch element per core.

v3 changes (driven by v2's ntff profile: the whole kernel is strung out
behind a single HWDGE DMA queue moving 5.9 MB at ~232 GB/s):
  * x ships as int8 (1 MB instead of 2 MB bf16) and is cast to bf16 by the
    SWDGE datapath during the DMA.  The dequant scale s_x folds into the
    host-packed Wq (and the softmax-denominator column constant c1), so the
    on-chip program is unchanged.  Simulated end-to-end absmax rel err:
    0.0126 (budget 0.02).  Weight tensors stay bf16 - int8 there blows the
    error budget (sim: >=0.018).
  * all inputs ride ONE ordered SWDGE (gpsimd) queue: cond/Wcond chunks
    first (phase-1 kv matmuls accumulate per-chunk as data lands), then
    Wq|Wout, then the four x chunks.  Outputs ride the sync HWDGE queue,
    so the two streams interleave on the 16 SDMA engines instead of
    FIFO-serializing behind each other.
  * softmax denominator fused into the context matmul via a ones-column
    that actually holds c1 = bf16(1/s_x): Z comes out pre-scaled and the
    1/Z normalization then applies the whole dequant for free.
  * warmup junk matmuls are N=128 (107 ns cold) instead of N=512, so the
    PE FIFO backlog when real work arrives is ~0.1 us, not ~3 us.
  * memsets moved off gpsimd - its Q7 cores must pump SWDGE descriptors
    without interruption.
"""

import os

import numpy as np

B = 8
C = 256
N_SPATIAL = 4096  # 64*64
P = 128
N_CORES = 8

WARM = int(os.environ.get("KERNEL_WARM", "18"))  # PE warmup matmuls at head
WARM_MID = int(os.environ.get("KERNEL_WARM_MID", "0"))  # mid-chain keepalive

_CACHE = {}
LAST_RESULTS = None  # BassKernelResults of the most recent run (for test.py)


def _build_nc(c1: float):
    import concourse.bacc as bacc
    import concourse.mybir as mybir
    import concourse.tile as tile

    fp32 = mybir.dt.float32
    bf16 = mybir.dt.bfloat16
    i8 = mybir.dt.int8
    AF = mybir.ActivationFunctionType

    nc = bacc.Bacc("TRN2", target_bir_lowering=False, debug=False,
                   num_devices=N_CORES)

    # wkv:   chunk j on axis 1: [cf_j (128) | wck_j (512) | wcv_j (512)]
    # wqo:   wq_perm (4x256) + woT_perm (4x256)            -> [128, 2048]
    # xq:    x int8, interleaved c=2p+ck, 4 col-chunks     -> [128,4,2,1024]
    # bias2: b_out[2p+mo]                                  -> [128, 2] f32
    # outp:  out rows 2p+mo, 8 col-chunks of 512           -> [128,8,2,512]
    wkv_t = nc.dram_tensor("wkv", [P, 4, 1152], bf16, kind="ExternalInput").ap()
    wqo_t = nc.dram_tensor("wqo", [P, 2048], bf16, kind="ExternalInput").ap()
    xq_t = nc.dram_tensor("xq", [P, 4, 2, 1024], i8, kind="ExternalInput").ap()
    bias_t = nc.dram_tensor("bias2", [P, 2], fp32, kind="ExternalInput").ap()
    out_t = nc.dram_tensor("out", [P, 8, 2, 512], bf16, kind="ExternalOutput").ap()

    with tile.TileContext(nc) as tc:
        with (
            tc.tile_pool(name="main", bufs=1) as mainp,
            tc.tile_pool(name="work", bufs=2) as workp,
            tc.tile_pool(name="outp", bufs=6) as outp,
            tc.tile_pool(name="ps", bufs=2, space="PSUM") as psp,
            tc.tile_pool(name="psA", bufs=3, space="PSUM") as psA,
            tc.tile_pool(name="psO", bufs=3, space="PSUM") as psO,
        ):
            # --- junk-matmul operand, first so the PE can warm immediately
            wl = mainp.tile([P, 128], bf16)
            nc.vector.memset(wl, 0.0)

            # --- input DMA triggers, critical-path order, ALL on the SWDGE
            # (gpsimd) queue so they stream strictly in this order while the
            # sync HWDGE queue stays free for the output tiles.
            wkv_a = mainp.tile([P, 2, 1152], bf16)
            nc.gpsimd.dma_start(wkv_a, wkv_t[:, 0:2, :])
            wkv_b = mainp.tile([P, 2, 1152], bf16)
            nc.gpsimd.dma_start(wkv_b, wkv_t[:, 2:4, :])
            wqo_sb = mainp.tile([P, 2048], bf16)
            nc.gpsimd.dma_start(wqo_sb, wqo_t)
            bias_sb = mainp.tile([P, 2], fp32)
            nc.gpsimd.dma_start(bias_sb, bias_t)
            x_sb = []
            for cc in range(4):
                t = mainp.tile([P, 2, 1024], bf16, tag=f"x{cc}")
                nc.gpsimd.dma_start(t, xq_t[:, cc, :, :])  # int8 -> bf16 cast
                x_sb.append(t)

            # persistent SBUF tiles + fills (NOT on gpsimd - its Q7 cores
            # are busy emitting DMA descriptors)
            vTo = mainp.tile([P, 4, 129], bf16)
            nc.vector.memset(vTo[:, :, 128:129], c1)
            ctx_bd = mainp.tile([P, 4, 128], bf16)
            nc.vector.memset(ctx_bd, 0.0)

            def keep_warm(n):
                for _ in range(n):
                    pj = psO.tile([P, 512], fp32, tag="O")
                    nc.tensor.matmul(pj[:, 0:128], wl, wl, start=True, stop=True)

            keep_warm(WARM)

            # --- phase 1: per-batch W_comb (256x256) ---
            # kvT k-half and v-half accumulate per wkv chunk as it lands.
            pkv = psp.tile([P, 512], fp32, tag="p1")
            pvv = psp.tile([P, 4, 128], fp32, tag="p1")
            for j in range(4):
                ch = (wkv_a if j < 2 else wkv_b)[:, j % 2, :]
                nc.tensor.matmul(pkv, ch[0:128, 0:128], ch[:, 128:640],
                                 start=(j == 0), stop=(j == 3))
                nc.tensor.matmul(pvv, ch[0:128, 0:128], ch[:, 640:1152],
                                 start=(j == 0), stop=(j == 3))
            expkT = mainp.tile([P, 512], bf16)
            nc.scalar.activation(out=expkT, in_=pkv, func=AF.Exp)
            nc.vector.tensor_copy(out=vTo[:, 0:2, 0:128], in_=pvv[:, 0:2, :])
            nc.scalar.activation(out=vTo[:, 2:4, 0:128], in_=pvv[:, 2:4, :],
                                 func=AF.Identity)
            keep_warm(WARM_MID)

            # fused context + softmax denominator per head pair i:
            #   pc_i[:, 0:128] = expkT_i^T @ vT_i ; pc_i[:, 128] = c1*Z
            # then extract diagonal 64x64 blocks scaled by 1/(c1*Z) (low half
            # on vector, high half on scalar), pipelined with the A matmuls.
            pcs = []
            for i in range(4):
                pc = psA.tile([P, 129], fp32, tag="pA")
                nc.tensor.matmul(pc, expkT[:, 128 * i:128 * (i + 1)],
                                 vTo[:, i, :], start=True, stop=True)
                pcs.append(pc)
            A_sb = mainp.tile([P, 4, 256], bf16)
            for i in range(4):
                rc = workp.tile([P, 1], fp32, tag=f"r{i}")
                nc.vector.reciprocal(rc, pcs[i][:, 128:129])
                nc.vector.tensor_scalar_mul(ctx_bd[0:64, i, 0:64],
                                            pcs[i][0:64, 0:64], rc[0:64])
                nc.scalar.activation(out=ctx_bd[64:128, i, 64:128],
                                     in_=pcs[i][64:128, 64:128],
                                     func=AF.Identity, scale=rc[64:128])
                pa = psA.tile([P, 256], fp32, tag="pA")
                nc.tensor.matmul(pa, ctx_bd[:, i, :],
                                 wqo_sb[:, 256 * i:256 * (i + 1)],
                                 start=True, stop=True)
                if i % 2 == 0:
                    nc.vector.tensor_copy(out=A_sb[:, i, :], in_=pa)
                else:
                    nc.scalar.activation(out=A_sb[:, i, :], in_=pa,
                                         func=AF.Identity)

            # W_combT[c, o'] = sum_he A[he, c] * WoutT_perm[he, o']
            wo_off = 1024
            wc_sb = mainp.tile([P, 2, 256], bf16)
            for ck in range(2):
                pw = psA.tile([P, 256], fp32, tag="pA")
                for kk in range(4):
                    nc.tensor.matmul(pw, A_sb[:, kk, 128 * ck:128 * (ck + 1)],
                                     wqo_sb[:, wo_off + 256 * kk:wo_off + 256 * (kk + 1)],
                                     start=(kk == 0), stop=(kk == 3))
                if ck == 0:
                    nc.vector.tensor_copy(out=wc_sb[:, ck, :], in_=pw)
                else:
                    nc.scalar.activation(out=wc_sb[:, ck, :], in_=pw,
                                         func=AF.Identity)

            # --- phase 2: OUT = W_comb @ xf + bias, streamed over x chunks.
            # The last spatial tile is split in half so the final
            # compute->store->drain tail is shorter.
            tiles = [(nt, 0, 512) for nt in range(7)]
            tiles += [(7, 0, 256), (7, 256, 256)]
            for nt, c0, cw in tiles:
                cc, sub = nt // 2, nt % 2
                ot = outp.tile([P, 2, 512], bf16, tag="osb")
                for mo in range(2):
                    po = psO.tile([P, 512], fp32, tag="O")
                    for ck in range(2):
                        nc.tensor.matmul(
                            po[:, 0:cw], wc_sb[:, ck, 128 * mo:128 * (mo + 1)],
                            x_sb[cc][:, ck, 512 * sub + c0:512 * sub + c0 + cw],
                            start=(ck == 0), stop=(ck == 1))
                    if mo == 0:
                        nc.scalar.activation(out=ot[:, mo, 0:cw], in_=po[:, 0:cw],
                                             func=AF.Identity,
                                             bias=bias_sb[:, 0:1], scale=1.0)
                    else:
                        nc.vector.tensor_scalar_add(out=ot[:, mo, 0:cw],
                                                    in0=po[:, 0:cw],
                                                    scalar1=bias_sb[:, 1:2])
                nc.sync.dma_start(out_t[:, nt, :, c0:c0 + cw], ot[:, :, 0:cw])

    nc.compile()
    return nc


def kernel(x, cond_emb, Wq, Wcond, Wout, b_out):
    import ml_dtypes
    from concourse.bass_utils import run_bass_kernel_spmd

    global LAST_RESULTS

    bf = ml_dtypes.bfloat16

    # --- x int8 quantization; scale folds into Wq and the c1 column ---
    s_x = max(float(np.abs(x).max()), 1e-30) / 127.0
    c1 = float(np.float32(bf(1.0 / s_x)))  # bf16-exact denominator constant
    key = ("nc", c1)
    if key not in _CACHE:
        _CACHE.clear()
        _CACHE[key] = _build_nc(c1)
    nc = _CACHE[key]

    # cf chunks: cf[j*128+p, m] -> [p, j, m]
    cf = np.asarray(cond_emb, np.float32).reshape(B, 4, P, P)
    cf_p = np.transpose(cf, (0, 2, 1, 3))  # [B, P, 4, 128]
    # wcondT chunks: wct[j*128+p, o] -> [p, j, o]
    wct = np.ascontiguousarray(np.asarray(Wcond, np.float32).T).reshape(4, P, 1024)
    wckv = np.transpose(wct, (1, 0, 2))  # [P, 4, 1024] (k cols 0:512, v 512:1024)
    # Wq scaled by c1*s_x (exact inverse of the on-chip 1/(c1*Z) dequant),
    # with columns parity-permuted (c = 2j+ck -> block ck, col j)
    wq_w = np.asarray(Wq, np.float32) * (c1 * s_x)
    wq_perm = wq_w.reshape(512, P, 2)
    wq_perm = np.transpose(wq_perm, (0, 2, 1)).reshape(512, 256)
    wq_p = np.transpose(wq_perm.reshape(4, P, 256), (1, 0, 2)).reshape(P, 1024)
    # WoutT with columns parity-permuted (o = 2i+mo -> block mo, col i)
    woT = np.ascontiguousarray(np.asarray(Wout, np.float32).T)
    woT_perm = np.transpose(woT.reshape(512, P, 2), (0, 2, 1)).reshape(512, 256)
    wo_p = np.transpose(woT_perm.reshape(4, P, 256), (1, 0, 2)).reshape(P, 1024)

    wkv = np.empty((B, P, 4, 1152), bf)
    wkv[:, :, :, 0:128] = cf_p.astype(bf)
    wkv[:, :, :, 128:1152] = wckv[None].astype(bf)
    wqo = np.empty((P, 2048), bf)
    wqo[:, 0:1024] = wq_p.astype(bf)
    wqo[:, 1024:2048] = wo_p.astype(bf)
    wqo = np.broadcast_to(wqo, (B, P, 2048))
    # x int8, interleaved: xq[p, cc, ck, u] = round(x[b, 2p+ck, cc*1024+u]/s_x)
    xr = np.asarray(x, np.float32).reshape(B, P, 2, 4, 1024)
    xq = np.clip(np.round(xr / s_x), -127, 127).astype(np.int8)
    xq = np.transpose(xq, (0, 1, 3, 2, 4))  # [B, P, 4, 2, 1024]
    bias2 = np.ascontiguousarray(
        np.asarray(b_out, np.float32).reshape(P, 2))

    in_maps = [
        {
            "wkv": np.ascontiguousarray(wkv[b]),
            "wqo": np.ascontiguousarray(wqo[b]),
            "xq": np.ascontiguousarray(xq[b]),
            "bias2": bias2,
        }
        for b in range(B)
    ]

    trace = bool(int(os.environ.get("KERNEL_TRACE", "0")))
    res = run_bass_kernel_spmd(nc, in_maps, core_ids=list(range(N_CORES)),
                               trace=trace)
    LAST_RESULTS = res
    # out_p[p, nt, mo, u] = out[b, 2p+mo, nt*512+u]
    outs = []
    for b in range(B):
        arr = np.asarray(res.results[b]["out"]).astype(np.float32)
        outs.append(np.transpose(arr, (0, 2, 1, 3)).reshape(C, N_SPATIAL))
    return np.stack(outs).reshape(B, C, 64, 64)


if __name__ == "__main__":
    xs = np.random.RandomState(0)
    ins = {
        "x": xs.randn(8, 256, 64, 64).astype(np.float32),
        "cond_emb": xs.randn(8, 512, 1, 128).astype(np.float32),
        "Wq": (xs.randn(512, 256) * 0.05).astype(np.float32),
        "Wcond": (xs.randn(1024, 512) * 0.05).astype(np.float32),
        "Wout": (xs.randn(256, 512) * 0.05).astype(np.float32),
        "b_out": np.zeros(256, np.float32),
    }
    o = kernel(**ins)
    print("ran, shape", o.shape)
